# revision 1
# baseline (speedup 1.0000x reference)
"""ChebConv (K=5) Trainium2 kernel, 8 NeuronCores.

Strategy (node sharding):
  - Nodes are sharded across 8 cores (5120 rows/core, padded N=40960).
  - Each SpMM stage: per-core dma_gather of source rows by edge col index,
    then TensorE matmuls with host-precomputed lap-weighted indicator
    matrices M[e, n] = 2*lap(e) accumulate segment sums in PSUM.
  - Chebyshev recursion T_k = 2 L T_{k-1} - T_{k-2} is realized as
    PSUM accumulation: sum_t M2[t].T @ G[t] + (-I).T @ T_{k-2}.
  - After each stage an AllGather rebuilds the full node table for the
    next stage's gathers.
  - GEMM out += T_k @ W_k is fused per node tile: T_k tiles are
    re-loaded transposed via DMA-transpose and fed to TensorE.
  - Everything bf16 on the wire / fp32 in PSUM.

Host-side prep computes degrees, Laplacian edge values, the slot
permutation (slots sorted by edge count so the shared SPMD profile is
tight), lo/hi gather split (int16 index limit), and all index/indicator
tensors.
"""

import os
import numpy as np
import ml_dtypes

bf16 = ml_dtypes.bfloat16

CFG_FULL = dict(
    N=40000, C=256, K=5, NC=8, TILE=128,
    NPAD=40960, SHARD=5120, HALF=32768, TBMAX=40,
)

LAST_EXEC_NS = None
_PROGRAM_CACHE = {}


# --------------------------------------------------------------------------
# host preprocessing
# --------------------------------------------------------------------------

def _prep(x, edge_index, edge_weight, cfg):
    N, C, K = cfg["N"], cfg["C"], cfg["K"]
    NC, TILE = cfg["NC"], cfg["TILE"]
    NPAD, SHARD, HALF = cfg["NPAD"], cfg["SHARD"], cfg["HALF"]
    NT = SHARD // TILE
    NWIN = NPAD // TILE

    row = np.asarray(edge_index[0], dtype=np.int64)
    col = np.asarray(edge_index[1], dtype=np.int64)
    ew = np.asarray(edge_weight, dtype=np.float32)
    keep = row != col
    deg = np.bincount(row[keep], minlength=N).astype(np.float32)
    dis = np.where(deg > 0, 1.0 / np.sqrt(np.maximum(deg, 1.0)), 0.0).astype(np.float32)
    lap = (-dis[row] * np.where(keep, ew, 0.0) * dis[col]).astype(np.float32)
    nz = lap != 0.0
    row, col, lap = row[nz], col[nz], lap[nz]
    m2val = 2.0 * lap  # stage-1 copy uses scale 0.5 to undo the factor 2

    # slot permutation: per core, windows sorted by edge count descending
    win = row // TILE
    wcnt = np.bincount(win, minlength=NWIN)
    perm = np.zeros((NC, NT), dtype=np.int64)
    for c in range(NC):
        wins = np.arange(c * NT, (c + 1) * NT)
        perm[c] = wins[np.argsort(-wcnt[wins], kind="stable")]
    slotpos = np.zeros(NWIN, dtype=np.int64)
    for c in range(NC):
        slotpos[perm[c]] = np.arange(NT)
    nodes = np.arange(N)
    pos = (nodes // SHARD) * SHARD + slotpos[nodes // TILE] * TILE + nodes % TILE

    tcol = pos[col]                  # table position of each edge's source
    islo = tcol < HALF
    core_of = row // SHARD
    slot_of = slotpos[win]
    nloc = row % TILE                # local dst row within its tile

    # per (core, slot) lo/hi counts -> shared profile
    keyc = core_of * NT + slot_of
    lo_cnt = np.bincount(keyc[islo], minlength=NC * NT).reshape(NC, NT)
    hi_cnt = np.bincount(keyc[~islo], minlength=NC * NT).reshape(NC, NT)
    L = np.maximum((-(-lo_cnt // TILE)).max(axis=0), 1)
    H = (-(-hi_cnt // TILE)).max(axis=0)
    assert (lo_cnt <= L[None, :] * TILE).all() and (hi_cnt <= H[None, :] * TILE).all()
    T = L + H
    NSUB = int(T.sum())

    # batches of consecutive slots with sum(T) <= TBMAX
    TBMAX = cfg["TBMAX"]
    batches = []  # (j0, j1_excl, gstart, nlo, nhi)
    j = 0
    gstart = 0
    while j < NT:
        j1 = j
        tot = 0
        while j1 < NT and tot + T[j1] <= TBMAX:
            tot += T[j1]
            j1 += 1
        nlo = int(L[j:j1].sum())
        nhi = int(H[j:j1].sum())
        batches.append((j, j1, gstart, nlo, nhi))
        j = j1
        gstart += nlo + nhi
    assert gstart == NSUB

    # G-column (== M subtile index) of (slot j, subtile t):
    # within a batch, lo runs of all slots first, then hi runs.
    gcol_lo = np.zeros(NT, dtype=np.int64)   # first lo column of slot j
    gcol_hi = np.zeros(NT, dtype=np.int64)
    for (j0, j1, gs, nlo, nhi) in batches:
        o = gs
        for j in range(j0, j1):
            gcol_lo[j] = o
            o += L[j]
        for j in range(j0, j1):
            gcol_hi[j] = o
            o += H[j]

    # per-core M tiles and gather indices
    m_tiles_all, idx_all = [], []
    for c in range(NC):
        m = np.zeros((NSUB, TILE, TILE), dtype=np.float32)
        idx = np.zeros(NSUB * TILE, dtype=np.int64)  # table idx per gather row
        sel_c = core_of == c
        for j in range(NT):
            sel = sel_c & (slot_of == j)
            for hi in (False, True):
                s = sel & (islo != hi)
                tc_, nl_, va_ = tcol[s], nloc[s], m2val[s]
                o = np.argsort(tc_, kind="stable")
                tc_, nl_, va_ = tc_[o], nl_[o], va_[o]
                base = gcol_hi[j] if hi else gcol_lo[j]
                nsub = H[j] if hi else L[j]
                e = np.arange(tc_.size)
                sub = base + e // TILE
                erow = e % TILE
                m[sub, erow, nl_] = va_
                gi = tc_ - (HALF if hi else 0)
                idx[base * TILE:(base + nsub) * TILE] = (HALF - HALF) if hi else 0
                idx[base * TILE + e] = gi
                # padding rows keep idx 0 (valid row, M row is zero)
        m_tiles_all.append(m.astype(bf16).reshape(NSUB * TILE, TILE))
        # wrap idx into [128, NSUB*8] int16: row i -> (partition i%16, col i//16),
        # replicated over the 8 partition groups
        iw = idx.reshape(NSUB, TILE // 16, 16).astype(np.int16)  # [sub, 8, 16]
        arr = np.zeros((128, NSUB * (TILE // 16)), dtype=np.int16)
        cols = iw.transpose(0, 1, 2).reshape(NSUB * (TILE // 16), 16)  # col-major over (sub, s)
        for rep in range(8):
            arr[rep * 16:(rep + 1) * 16, :] = cols.T
        idx_all.append(arr)

    # tables
    x = np.asarray(x, dtype=np.float32)
    x_table = np.zeros((NPAD, C), dtype=bf16)
    x_table[pos] = x.astype(bf16)

    meta = dict(L=L, H=H, T=T, NSUB=NSUB, batches=batches,
                gcol_lo=gcol_lo, gcol_hi=gcol_hi, pos=pos)
    return m_tiles_all, idx_all, x_table, meta


# --------------------------------------------------------------------------
# device program
# --------------------------------------------------------------------------

def _build(cfg, meta, mode="full"):
    import concourse.bacc as bacc
    import concourse.mybir as mybir
    import concourse.tile as tile
    from concourse.library_config import mlp
    do_gemm = mode in ("full", "nospmm", "noag")
    do_spmm = mode in ("full", "nogemm", "noag")

    C, K, NC, TILE = cfg["C"], cfg["K"], cfg["NC"], cfg["TILE"]
    NPAD, SHARD, HALF, TBMAX = cfg["NPAD"], cfg["SHARD"], cfg["HALF"], cfg["TBMAX"]
    NT = SHARD // TILE
    L, H, T = meta["L"], meta["H"], meta["T"]
    NSUB, batches = meta["NSUB"], meta["batches"]
    gcol_lo, gcol_hi = meta["gcol_lo"], meta["gcol_hi"]
    IDXW = TILE // 16

    nc = bacc.Bacc("TRN2", target_bir_lowering=False, debug=False,
                   num_devices=NC)
    dt = mybir.dt
    x_table = nc.dram_tensor("x_table", [NPAD, C], dt.bfloat16, kind="ExternalInput")
    x_shard = nc.dram_tensor("x_shard", [SHARD, C], dt.bfloat16, kind="ExternalInput")
    m_in = nc.dram_tensor("m_tiles", [NSUB * TILE, TILE], dt.bfloat16, kind="ExternalInput")
    idx_in = nc.dram_tensor("idxs", [128, NSUB * IDXW], dt.int16, kind="ExternalInput")
    w_in = nc.dram_tensor("w_chunks", [2 * K * 128, C], dt.bfloat16, kind="ExternalInput")
    bias_in = nc.dram_tensor("bias_bcast", [128, C], dt.float32, kind="ExternalInput")
    negi_in = nc.dram_tensor("neg_id", [128, 128], dt.bfloat16, kind="ExternalInput")
    out_t = nc.dram_tensor("out_shard", [SHARD, C], dt.float32, kind="ExternalOutput")

    with tile.TileContext(nc) as tc:
        nc.gpsimd.load_library(mlp)
        with (
            tc.tile_pool(name="const", bufs=1) as const,
            tc.tile_pool(name="acc", bufs=NT) as accp,
            tc.tile_pool(name="g", bufs=2) as gp,
            tc.tile_pool(name="tn", bufs=4) as tnp,
            tc.tile_pool(name="tp", bufs=4) as tpp,
            tc.tile_pool(name="tt", bufs=6) as ttp,
            tc.tile_pool(name="sp", bufs=3, space="PSUM") as spp,
            tc.tile_pool(name="gp", bufs=3, space="PSUM") as gpp,
            tc.tile_pool(name="dram", bufs=1, space="DRAM") as dram,
        ):
            m_sb = const.tile([128, NSUB, TILE], dt.bfloat16)
            nc.sync.dma_start(m_sb[:], m_in[:].rearrange("(s p) n -> p s n", p=128))
            idx_sb = const.tile([128, NSUB * IDXW], dt.int16)
            nc.sync.dma_start(idx_sb[:], idx_in[:])
            w_sb = const.tile([128, 2 * K, C], dt.bfloat16)
            nc.sync.dma_start(w_sb[:], w_in[:].rearrange("(w p) n -> p w n", p=128))
            bias_sb = const.tile([128, C], dt.float32)
            nc.sync.dma_start(bias_sb[:], bias_in[:])
            negi_sb = const.tile([128, 128], dt.bfloat16)
            nc.sync.dma_start(negi_sb[:], negi_in[:])

            S = {}      # gather tables per stage (stage k gathers from S[k])
            B = {}      # per-core shard of T_k (AG input / local reload)
            B[0] = x_shard
            S[1] = x_table
            for k in range(2, K):
                S[k] = dram.tile([NPAD, C], dt.bfloat16, addr_space="Shared",
                                 name=f"s_table_{k}")
            for k in range(1, K):
                B[k] = dram.tile([SHARD, C], dt.bfloat16, name=f"b_shard_{k}")

            acc = []
            for j in range(NT):
                a = accp.tile([128, C], dt.float32, tag="acc", name=f"acc_{j}")
                acc.append(a)

            # DMA-transpose loads hang the chip if they overlap a collective
            # (xbar-mode HW bug; Tile does not serialize this pair in this
            # build). Track them and make every AllGather depend on all
            # transposes emitted so far.
            transposes = []

            def gemm(j, k, src_rows):
                """acc[j] (+)= T_k[tile j] @ W_k  (+ bias at k==0)."""
                if not do_gemm:
                    if k == 0:
                        nc.vector.tensor_copy(acc[j][:], bias_sb[:])
                    return
                gps = gpp.tile([128, C], dt.float32, tag="gps")
                for ch in range(2):
                    tT = ttp.tile([128, 128], dt.bfloat16, tag="tt")
                    tr = nc.sync.dma_start(
                        tT[:], src_rows[:, ch * 128:(ch + 1) * 128], transpose=True)
                    transposes.append(tr)
                    nc.tensor.matmul(gps[:], lhsT=tT[:], rhs=w_sb[:, 2 * k + ch, :],
                                     start=(ch == 0), stop=(ch == 1))
                if k == 0:
                    nc.vector.tensor_add(acc[j][:], gps[:], bias_sb[:])
                else:
                    nc.vector.tensor_add(acc[j][:], acc[j][:], gps[:])

            # stage 0: out = x @ W0 + bias
            for j in range(NT):
                gemm(j, 0, x_shard[j * TILE:(j + 1) * TILE, :])

            # stages 1..K-1
            for k in range(1, K if do_spmm else 1):
                src = S[k]
                for (j0, j1, gstart, nlo, nhi) in batches:
                    nb = nlo + nhi
                    g = gp.tile([128, TBMAX, C], dt.bfloat16, tag="g")
                    # SWDGE descriptor ring holds ~1024 descriptors; one
                    # gather call emits one descriptor per index, so cap
                    # calls at SUBCAP subtiles (SUBCAP*128 indices).
                    SUBCAP = 8
                    segs = ([(o, min(SUBCAP, nlo - o), src[0:HALF, :])
                             for o in range(0, nlo, SUBCAP)] +
                            [(nlo + o, min(SUBCAP, nhi - o), src[HALF:NPAD, :])
                             for o in range(0, nhi, SUBCAP)])
                    for (o, n, s_ap) in segs:
                        nc.gpsimd.dma_gather(
                            g[:, o:o + n, :], s_ap,
                            idx_sb[:, (gstart + o) * IDXW:(gstart + o + n) * IDXW],
                            n * TILE, n * TILE, C)
                    for j in range(j0, j1):
                        psum = spp.tile([128, C], dt.float32, tag="sp")
                        subs = ([gcol_lo[j] + t for t in range(L[j])] +
                                [gcol_hi[j] + t for t in range(H[j])])
                        for ti, s in enumerate(subs):
                            nc.tensor.matmul(
                                psum[:], lhsT=m_sb[:, s, :], rhs=g[:, s - gstart, :],
                                start=(ti == 0),
                                stop=(k == 1 and ti == len(subs) - 1))
                        if k > 1:
                            tprev = tpp.tile([128, C], dt.bfloat16, tag="tp")
                            nc.sync.dma_start(
                                tprev[:], B[k - 2][j * TILE:(j + 1) * TILE, :])
                            nc.tensor.matmul(psum[:], lhsT=negi_sb[:], rhs=tprev[:],
                                             start=False, stop=True)
                        tnew = tnp.tile([128, C], dt.bfloat16, tag="tn")
                        nc.scalar.activation(tnew[:], psum[:],
                                             mybir.ActivationFunctionType.Copy,
                                             scale=(0.5 if k == 1 else 1.0))
                        nc.sync.dma_start(B[k][j * TILE:(j + 1) * TILE, :], tnew[:])
                        gemm(j, k, B[k][j * TILE:(j + 1) * TILE, :])
                if k < K - 1 and mode != "noag":
                    ag = nc.gpsimd.collective_compute(
                        "AllGather", mybir.AluOpType.bypass,
                        replica_groups=[list(range(NC))],
                        ins=[B[k][:].opt()], outs=[S[k + 1][:].opt()])
                    for tr in transposes:
                        tile.add_dep_helper(ag.ins, tr.ins,
                                            reason="xbar: AG after transposes")
                    transposes.clear()

            for j in range(NT):
                nc.sync.dma_start(out_t[j * TILE:(j + 1) * TILE, :], acc[j][:])
    nc.compile()
    return nc


# --------------------------------------------------------------------------
# entry point
# --------------------------------------------------------------------------

def kernel(x, edge_index, edge_weight, weight, bias):
    global LAST_EXEC_NS
    from concourse.bass_utils import run_bass_kernel_spmd

    cfg = CFG_FULL
    N, C, K, NC, SHARD = cfg["N"], cfg["C"], cfg["K"], cfg["NC"], cfg["SHARD"]
    x = np.asarray(x)
    weight = np.asarray(weight, dtype=np.float32)
    bias = np.asarray(bias, dtype=np.float32)

    m_tiles_all, idx_all, x_table, meta = _prep(x, edge_index, edge_weight, cfg)

    key = (tuple(meta["L"]), tuple(meta["H"]))
    if key not in _PROGRAM_CACHE:
        _PROGRAM_CACHE[key] = _build(cfg, meta)
    nc = _PROGRAM_CACHE[key]

    w_chunks = np.zeros((2 * K * 128, C), dtype=bf16)
    for k in range(K):
        for ch in range(2):
            w_chunks[(2 * k + ch) * 128:(2 * k + ch + 1) * 128] = \
                weight[k, ch * 128:(ch + 1) * 128, :].astype(bf16)
    bias_bcast = np.broadcast_to(bias, (128, C)).astype(np.float32).copy()
    neg_id = (-np.eye(128, dtype=np.float32)).astype(bf16)

    in_maps = []
    for c in range(NC):
        in_maps.append({
            "x_table": x_table,
            "x_shard": x_table[c * SHARD:(c + 1) * SHARD],
            "m_tiles": m_tiles_all[c],
            "idxs": idx_all[c],
            "w_chunks": w_chunks,
            "bias_bcast": bias_bcast,
            "neg_id": neg_id,
        })

    trace = bool(os.environ.get("CHEB_TRACE"))
    kw = {}
    if trace:
        kw = dict(trace=True, tmpdir=os.environ.get("CHEB_TRACE_DIR") or None)
    res = run_bass_kernel_spmd(nc, in_maps, core_ids=list(range(NC)), **kw)
    LAST_EXEC_NS = res.exec_time_ns

    pos = meta["pos"]
    shards = [res.results[c]["out_shard"] for c in range(NC)]
    full = np.concatenate(shards, axis=0)      # [NPAD(table order), C]
    out = full[pos]                            # back to node order
    return np.ascontiguousarray(out.astype(np.float32))



# revision 17
# speedup vs baseline: 1.8973x; 1.8973x over previous
"""ChebConv (K=5) Trainium2 kernel, 8 NeuronCores.

Strategy (node sharding):
  - Nodes are sharded across 8 cores (5120 rows/core, padded N=40960).
  - Each SpMM stage: per-core dma_gather of source rows by edge col index,
    then TensorE matmuls with host-precomputed lap-weighted indicator
    matrices M[e, n] = 2*lap(e) accumulate segment sums in PSUM.
  - Chebyshev recursion T_k = 2 L T_{k-1} - T_{k-2} is realized as
    PSUM accumulation: sum_t M2[t].T @ G[t] + (-I).T @ T_{k-2}.
  - After each stage an AllGather rebuilds the full node table for the
    next stage's gathers.
  - GEMM out += T_k @ W_k is fused per node tile: T_k tiles are
    re-loaded transposed via DMA-transpose and fed to TensorE.
  - Everything bf16 on the wire / fp32 in PSUM.

Host-side prep computes degrees, Laplacian edge values, the slot
permutation (slots sorted by edge count so the shared SPMD profile is
tight), lo/hi gather split (int16 index limit), and all index/indicator
tensors.
"""

import os
import numpy as np
import ml_dtypes

bf16 = ml_dtypes.bfloat16

CFG_FULL = dict(
    N=40000, C=256, K=5, NC=8, TILE=128,
    NPAD=40960, SHARD=5120, HALF=20480, TBMAX=40,
)

LAST_EXEC_NS = None
_PROGRAM_CACHE = {}


# --------------------------------------------------------------------------
# host preprocessing
# --------------------------------------------------------------------------

def _prep(x, edge_index, edge_weight, cfg):
    N, C, K = cfg["N"], cfg["C"], cfg["K"]
    NC, TILE = cfg["NC"], cfg["TILE"]
    NPAD, SHARD, HALF = cfg["NPAD"], cfg["SHARD"], cfg["HALF"]
    NT = SHARD // TILE
    NWIN = NPAD // TILE

    row = np.asarray(edge_index[0], dtype=np.int64)
    col = np.asarray(edge_index[1], dtype=np.int64)
    ew = np.asarray(edge_weight, dtype=np.float32)
    keep = row != col
    deg = np.bincount(row[keep], minlength=N).astype(np.float32)
    dis = np.where(deg > 0, 1.0 / np.sqrt(np.maximum(deg, 1.0)), 0.0).astype(np.float32)
    lap = (-dis[row] * np.where(keep, ew, 0.0) * dis[col]).astype(np.float32)
    nz = lap != 0.0
    row, col, lap = row[nz], col[nz], lap[nz]
    m2val = 2.0 * lap  # stage-1 copy uses scale 0.5 to undo the factor 2

    # slot permutation: per core, windows sorted by edge count descending
    win = row // TILE
    wcnt = np.bincount(win, minlength=NWIN)
    perm = np.zeros((NC, NT), dtype=np.int64)
    for c in range(NC):
        wins = np.arange(c * NT, (c + 1) * NT)
        perm[c] = wins[np.argsort(-wcnt[wins], kind="stable")]
    slotpos = np.zeros(NWIN, dtype=np.int64)
    for c in range(NC):
        slotpos[perm[c]] = np.arange(NT)
    # table layout: [all cores' slots 0..19 | all cores' slots 20..39] so each
    # half-shard AllGather writes one contiguous table region
    HSH = SHARD // 2
    nodes = np.arange(N)
    core_n = nodes // SHARD
    local = slotpos[nodes // TILE] * TILE + nodes % TILE
    spos = core_n * SHARD + local           # position in concat-of-shards order
    pos = np.where(local < HSH,             # position in the gather table
                   core_n * HSH + local,
                   NC * HSH + core_n * HSH + (local - HSH))

    tcol = pos[col]                  # table position of each edge's source
    islo = tcol < HALF
    core_of = row // SHARD
    slot_of = slotpos[win]
    nloc = row % TILE                # local dst row within its tile

    # per (core, slot) lo/hi counts -> shared profile
    keyc = core_of * NT + slot_of
    lo_cnt = np.bincount(keyc[islo], minlength=NC * NT).reshape(NC, NT)
    hi_cnt = np.bincount(keyc[~islo], minlength=NC * NT).reshape(NC, NT)
    L = np.maximum((-(-lo_cnt // TILE)).max(axis=0), 1)
    H = (-(-hi_cnt // TILE)).max(axis=0)
    assert (lo_cnt <= L[None, :] * TILE).all() and (hi_cnt <= H[None, :] * TILE).all()
    T = L + H
    NSUB = int(T.sum())

    # batches of consecutive slots with sum(T) <= TBMAX
    TBMAX = cfg["TBMAX"]
    batches = []  # (j0, j1_excl, gstart, nlo, nhi)
    j = 0
    gstart = 0
    while j < NT:
        j1 = j
        tot = 0
        while j1 < NT and tot + T[j1] <= TBMAX:
            tot += T[j1]
            j1 += 1
        nlo = int(L[j:j1].sum())
        nhi = int(H[j:j1].sum())
        batches.append((j, j1, gstart, nlo, nhi))
        j = j1
        gstart += nlo + nhi
    assert gstart == NSUB

    # G-column (== M subtile index) of (slot j, subtile t):
    # within a batch, lo runs of all slots first, then hi runs.
    gcol_lo = np.zeros(NT, dtype=np.int64)   # first lo column of slot j
    gcol_hi = np.zeros(NT, dtype=np.int64)
    for (j0, j1, gs, nlo, nhi) in batches:
        o = gs
        for j in range(j0, j1):
            gcol_lo[j] = o
            o += L[j]
        for j in range(j0, j1):
            gcol_hi[j] = o
            o += H[j]

    # per-core M tiles and gather indices
    m_tiles_all, idx_all = [], []
    for c in range(NC):
        m = np.zeros((NSUB, TILE, TILE), dtype=np.float32)
        idx = np.zeros(NSUB * TILE, dtype=np.int64)  # table idx per gather row
        sel_c = core_of == c
        for j in range(NT):
            sel = sel_c & (slot_of == j)
            for hi in (False, True):
                s = sel & (islo != hi)
                tc_, nl_, va_ = tcol[s], nloc[s], m2val[s]
                o = np.argsort(tc_, kind="stable")
                tc_, nl_, va_ = tc_[o], nl_[o], va_[o]
                base = gcol_hi[j] if hi else gcol_lo[j]
                nsub = H[j] if hi else L[j]
                e = np.arange(tc_.size)
                sub = base + e // TILE
                erow = e % TILE
                m[sub, erow, nl_] = va_
                gi = tc_ - (HALF if hi else 0)
                idx[base * TILE:(base + nsub) * TILE] = (HALF - HALF) if hi else 0
                idx[base * TILE + e] = gi
                # padding rows keep idx 0 (valid row, M row is zero)
        m_tiles_all.append(m.astype(bf16).reshape(NSUB * TILE, TILE))
        # wrap idx into [128, NSUB*8] int16: row i -> (partition i%16, col i//16),
        # replicated over the 8 partition groups
        iw = idx.reshape(NSUB, TILE // 16, 16).astype(np.int16)  # [sub, 8, 16]
        arr = np.zeros((128, NSUB * (TILE // 16)), dtype=np.int16)
        cols = iw.transpose(0, 1, 2).reshape(NSUB * (TILE // 16), 16)  # col-major over (sub, s)
        for rep in range(8):
            arr[rep * 16:(rep + 1) * 16, :] = cols.T
        idx_all.append(arr)

    # tables
    x = np.asarray(x, dtype=np.float32)
    x_table = np.zeros((NPAD, C), dtype=bf16)
    x_table[pos] = x.astype(bf16)
    x_shards = np.zeros((NPAD, C), dtype=bf16)
    x_shards[spos] = x.astype(bf16)

    meta = dict(L=L, H=H, T=T, NSUB=NSUB, batches=batches,
                gcol_lo=gcol_lo, gcol_hi=gcol_hi, pos=pos, spos=spos)
    return m_tiles_all, idx_all, x_table, x_shards, meta


# --------------------------------------------------------------------------
# device program
# --------------------------------------------------------------------------

def _build(cfg, meta, mode="full"):
    import concourse.bacc as bacc
    import concourse.mybir as mybir
    import concourse.tile as tile
    from concourse.library_config import mlp
    do_gemm = mode in ("full", "nospmm", "noag")
    do_spmm = mode in ("full", "nogemm", "noag")

    C, K, NC, TILE = cfg["C"], cfg["K"], cfg["NC"], cfg["TILE"]
    NPAD, SHARD, HALF, TBMAX = cfg["NPAD"], cfg["SHARD"], cfg["HALF"], cfg["TBMAX"]
    NT = SHARD // TILE
    L, H, T = meta["L"], meta["H"], meta["T"]
    NSUB, batches = meta["NSUB"], meta["batches"]
    gcol_lo, gcol_hi = meta["gcol_lo"], meta["gcol_hi"]
    IDXW = TILE // 16

    NQ = 4
    nc = bacc.Bacc("TRN2", target_bir_lowering=False, debug=False,
                   num_devices=NC, num_swdge_queues=NQ)
    dt = mybir.dt
    x_table = nc.dram_tensor("x_table", [NPAD, C], dt.bfloat16, kind="ExternalInput")
    x_shard = nc.dram_tensor("x_shard", [SHARD, C], dt.bfloat16, kind="ExternalInput")
    m_in = nc.dram_tensor("m_tiles", [NSUB * TILE, TILE], dt.bfloat16, kind="ExternalInput")
    idx_in = nc.dram_tensor("idxs", [128, NSUB * IDXW], dt.int16, kind="ExternalInput")
    w_in = nc.dram_tensor("w_chunks", [2 * K * 128, C], dt.bfloat16, kind="ExternalInput")
    bias_in = nc.dram_tensor("bias_bcast", [128, C], dt.float32, kind="ExternalInput")
    negi_in = nc.dram_tensor("neg_id", [128, 128], dt.bfloat16, kind="ExternalInput")
    posi_in = nc.dram_tensor("pos_id", [128, 128], dt.bfloat16, kind="ExternalInput")
    out_t = nc.dram_tensor("out_shard", [SHARD, C], dt.float32, kind="ExternalOutput")
    HSH = SHARD // 2

    with tile.TileContext(nc) as tc:
        nc.gpsimd.load_library(mlp)
        with (
            tc.tile_pool(name="const", bufs=1) as const,
            tc.tile_pool(name="acc", bufs=NT) as accp,
            tc.tile_pool(name="g", bufs=2) as gp,
            tc.tile_pool(name="tn", bufs=4) as tnp,
            tc.tile_pool(name="tp", bufs=4) as tpp,
            tc.tile_pool(name="tt", bufs=6) as ttp,
            tc.tile_pool(name="sp", bufs=3, space="PSUM") as spp,
            tc.tile_pool(name="gp", bufs=3, space="PSUM") as gpp,
            tc.tile_pool(name="tr", bufs=2, space="PSUM") as trp,
            tc.tile_pool(name="dram", bufs=1, space="DRAM") as dram,
        ):
            m_sb = const.tile([128, NSUB, TILE], dt.bfloat16)
            nc.sync.dma_start(m_sb[:], m_in[:].rearrange("(s p) n -> p s n", p=128))
            idx_sb = const.tile([128, NSUB * IDXW], dt.int16)
            nc.sync.dma_start(idx_sb[:], idx_in[:])
            w_sb = const.tile([128, 2 * K, C], dt.bfloat16)
            nc.sync.dma_start(w_sb[:], w_in[:].rearrange("(w p) n -> p w n", p=128))
            bias_sb = const.tile([128, C], dt.float32)
            nc.sync.dma_start(bias_sb[:], bias_in[:])
            negi_sb = const.tile([128, 128], dt.bfloat16)
            nc.sync.dma_start(negi_sb[:], negi_in[:])
            posi_sb = const.tile([128, 128], dt.bfloat16)
            nc.sync.dma_start(posi_sb[:], posi_in[:])
            gq_sems = [nc.alloc_semaphore(f"gq{q}") for q in range(NQ)]
            qrot = [0]

            # gather tables per stage: (lo, hi) halves are separate tensors so
            # each half-shard AllGather is the single writer of its tensor.
            S = {}
            B = {}      # per-core shard of T_k (AG input / local reload)
            B[0] = x_shard
            S[1] = (x_table[0:HALF, :], x_table[HALF:NPAD, :])
            for k in range(2, K):
                S[k] = (dram.tile([HALF, C], dt.bfloat16, addr_space="Shared",
                                  name=f"s_lo_{k}")[:],
                        dram.tile([NPAD - HALF, C], dt.bfloat16,
                                  addr_space="Shared", name=f"s_hi_{k}")[:])
            for k in range(1, K):
                B[k] = dram.tile([SHARD, C], dt.bfloat16, name=f"b_shard_{k}")

            acc = []
            for j in range(NT):
                a = accp.tile([128, C], dt.float32, tag="acc", name=f"acc_{j}")
                acc.append(a)

            def gemm(j, k, t_sb):
                """acc[j] (+)= T_k[tile j] @ W_k  (+ bias at k==0).

                t_sb: SBUF tile [128, C] bf16 holding T_k rows of tile j.
                Transposed via TensorE identity matmul (no DMA transpose)."""
                if not do_gemm:
                    if k == 0:
                        nc.vector.tensor_copy(acc[j][:], bias_sb[:])
                    return
                gps = gpp.tile([128, C], dt.float32, tag="gps")
                for ch in range(2):
                    pst = trp.tile([128, 128], dt.float32, tag="tr")
                    nc.tensor.matmul(pst[:], lhsT=t_sb[:, ch * 128:(ch + 1) * 128],
                                     rhs=posi_sb[:], start=True, stop=True)
                    tT = ttp.tile([128, 128], dt.bfloat16, tag="tt")
                    nc.vector.tensor_copy(tT[:], pst[:])
                    nc.tensor.matmul(gps[:], lhsT=tT[:], rhs=w_sb[:, 2 * k + ch, :],
                                     start=(ch == 0), stop=(ch == 1))
                if k == 0:
                    nc.vector.tensor_add(acc[j][:], gps[:], bias_sb[:])
                else:
                    nc.vector.tensor_add(acc[j][:], acc[j][:], gps[:])

            # stage 0: out = x @ W0 + bias
            for j in range(NT):
                xt = tnp.tile([128, C], dt.bfloat16, tag="tn")
                nc.sync.dma_start(xt[:], x_shard[j * TILE:(j + 1) * TILE, :])
                gemm(j, 0, xt)

            # stages 1..K-1
            for k in range(1, K if do_spmm else 1):
                src_lo, src_hi = S[k]
                for (j0, j1, gstart, nlo, nhi) in batches:
                    g = gp.tile([128, TBMAX, C], dt.bfloat16, tag="g")
                    # SWDGE descriptor ring holds ~1024 descriptors; one
                    # gather call emits one descriptor per index, so cap
                    # calls at SUBCAP subtiles (SUBCAP*128 indices).
                    # prepare_only + trigger decouples GPSIMD descriptor
                    # generation from the DMA transfer; rotate the 4 SWDGE
                    # queues so a queue's ring drains before its next prep.
                    SUBCAP = 7
                    segs = ([(o, min(SUBCAP, nlo - o), src_lo)
                             for o in range(0, nlo, SUBCAP)] +
                            [(nlo + o, min(SUBCAP, nhi - o), src_hi)
                             for o in range(0, nhi, SUBCAP)])
                    USE_PREP = bool(int(os.environ.get("CHEB_PREP", "0")))
                    for (o, n, s_ap) in segs:
                        q = qrot[0]
                        qrot[0] = (q + 1) % NQ
                        if USE_PREP:
                            nc.gpsimd.dma_gather(
                                g[:, o:o + n, :], s_ap,
                                idx_sb[:, (gstart + o) * IDXW:(gstart + o + n) * IDXW],
                                n * TILE, n * TILE, C,
                                prepare_only=True, sem=gq_sems[q], queue_num=q)
                            nc.gpsimd.trigger_dma(count=None, queue_num=q)
                        else:
                            nc.gpsimd.dma_gather(
                                g[:, o:o + n, :], s_ap,
                                idx_sb[:, (gstart + o) * IDXW:(gstart + o + n) * IDXW],
                                n * TILE, n * TILE, C, queue_num=q)
                    for j in range(j0, j1):
                        psum = spp.tile([128, C], dt.float32, tag="sp")
                        subs = ([gcol_lo[j] + t for t in range(L[j])] +
                                [gcol_hi[j] + t for t in range(H[j])])
                        for ti, s in enumerate(subs):
                            nc.tensor.matmul(
                                psum[:], lhsT=m_sb[:, s, :], rhs=g[:, s - gstart, :],
                                start=(ti == 0),
                                stop=(k == 1 and ti == len(subs) - 1))
                        if k > 1:
                            tprev = tpp.tile([128, C], dt.bfloat16, tag="tp")
                            nc.sync.dma_start(
                                tprev[:], B[k - 2][j * TILE:(j + 1) * TILE, :])
                            nc.tensor.matmul(psum[:], lhsT=negi_sb[:], rhs=tprev[:],
                                             start=False, stop=True)
                        tnew = tnp.tile([128, C], dt.bfloat16, tag="tn")
                        nc.scalar.activation(tnew[:], psum[:],
                                             mybir.ActivationFunctionType.Copy,
                                             scale=(0.5 if k == 1 else 1.0))
                        nc.sync.dma_start(B[k][j * TILE:(j + 1) * TILE, :], tnew[:])
                        gemm(j, k, tnew)
                        if k < K - 1 and mode != "noag" and j == NT // 2 - 1:
                            nc.gpsimd.collective_compute(
                                "AllGather", mybir.AluOpType.bypass,
                                replica_groups=[list(range(NC))],
                                ins=[B[k][0:HSH, :].opt()],
                                outs=[S[k + 1][0].opt()])
                if k < K - 1 and mode != "noag":
                    nc.gpsimd.collective_compute(
                        "AllGather", mybir.AluOpType.bypass,
                        replica_groups=[list(range(NC))],
                        ins=[B[k][HSH:SHARD, :].opt()],
                        outs=[S[k + 1][1].opt()])

            for j in range(NT):
                nc.sync.dma_start(out_t[j * TILE:(j + 1) * TILE, :], acc[j][:])
    nc.compile()
    return nc


# --------------------------------------------------------------------------
# entry point
# --------------------------------------------------------------------------

def kernel(x, edge_index, edge_weight, weight, bias):
    global LAST_EXEC_NS
    from concourse.bass_utils import run_bass_kernel_spmd

    cfg = CFG_FULL
    N, C, K, NC, SHARD = cfg["N"], cfg["C"], cfg["K"], cfg["NC"], cfg["SHARD"]
    x = np.asarray(x)
    weight = np.asarray(weight, dtype=np.float32)
    bias = np.asarray(bias, dtype=np.float32)

    m_tiles_all, idx_all, x_table, x_shards, meta = _prep(x, edge_index, edge_weight, cfg)

    key = (tuple(meta["L"]), tuple(meta["H"]))
    if key not in _PROGRAM_CACHE:
        _PROGRAM_CACHE[key] = _build(cfg, meta)
    nc = _PROGRAM_CACHE[key]

    w_chunks = np.zeros((2 * K * 128, C), dtype=bf16)
    for k in range(K):
        for ch in range(2):
            w_chunks[(2 * k + ch) * 128:(2 * k + ch + 1) * 128] = \
                weight[k, ch * 128:(ch + 1) * 128, :].astype(bf16)
    bias_bcast = np.broadcast_to(bias, (128, C)).astype(np.float32).copy()
    neg_id = (-np.eye(128, dtype=np.float32)).astype(bf16)
    pos_id = np.eye(128, dtype=np.float32).astype(bf16)

    in_maps = []
    for c in range(NC):
        in_maps.append({
            "x_table": x_table,
            "x_shard": x_shards[c * SHARD:(c + 1) * SHARD],
            "m_tiles": m_tiles_all[c],
            "idxs": idx_all[c],
            "w_chunks": w_chunks,
            "bias_bcast": bias_bcast,
            "neg_id": neg_id,
            "pos_id": pos_id,
        })

    trace = bool(os.environ.get("CHEB_TRACE"))
    kw = {}
    if trace:
        kw = dict(trace=True, tmpdir=os.environ.get("CHEB_TRACE_DIR") or None)
    res = run_bass_kernel_spmd(nc, in_maps, core_ids=list(range(NC)), **kw)
    LAST_EXEC_NS = res.exec_time_ns

    spos = meta["spos"]
    shards = [res.results[c]["out_shard"] for c in range(NC)]
    full = np.concatenate(shards, axis=0)      # [NPAD(shard order), C]
    out = full[spos]                           # back to node order
    return np.ascontiguousarray(out.astype(np.float32))

